# revision 1
# baseline (speedup 1.0000x reference)
"""Trainium2 Bass kernel for L0-regularized linear forward (hard-concrete gate).

Computes out[b,o] = sum_i x[b,i] * W[o,i] * z[b,o,i] + bias[o]
  where s = sigmoid((log(u) - log1p(-u) + log_alpha) / (2/3))
        z = clip(s * 1.2 - 0.1, 0, 1)

Shapes: x[32,2048] u[32,2048,2048] W[2048,2048] la[2048,2048] bias[2048]
Sharding: output-dim sharded, 2048/8 = 256 rows per core (each core reads
its slice of u/W/la/bias + full x; no collectives; concat outputs on host).

Per-core pipeline (o-tile layout [128 part, 2 halves, 2048 free]):
  ACT:  L1 = ln(u); L1 -= ln(1-u) via DVE; t = L1 + la (POOL); s = sigmoid(1.5 t)
  DVE:  z = clamp(1.2 s - 0.1, 0, 1); p = z * x_bcast;
        tensor_tensor_reduce: acc[o] = bias[o] + sum_i p * W   -> DMA to HBM
ACT table sets (ln vs sigmoid) are batched in groups of B_GROUP batches to
amortize the ~2.7us table switch.
"""

import sys
from contextlib import ExitStack

import numpy as np

if "/opt/trn_rl_repo" not in sys.path:
    sys.path.insert(0, "/opt/trn_rl_repo")

import concourse.bass as bass
import concourse.tile as tile
from concourse import bacc, mybir
from concourse.bass_utils import run_bass_kernel_spmd

F32 = mybir.dt.float32
F16 = mybir.dt.float16

B, OUT, IN = 32, 2048, 2048
N_CORES = 8
O_SH = OUT // N_CORES          # 256 output rows per core
H = O_SH // 128                # 2 partition-halves per core
B_GROUP = 8                    # batches per ACT-table-set phase

_CACHE = {}


def _build_nc(trace=False):
    key = ("nc", trace)
    if key in _CACHE:
        return _CACHE[key]

    nc = bacc.Bacc(
        "TRN2",
        target_bir_lowering=False,
        debug=False,
        num_devices=N_CORES,
    )
    x_d = nc.dram_tensor("x", [B, IN], F32, kind="ExternalInput").ap()
    u_d = nc.dram_tensor("u", [B, O_SH, IN], F32, kind="ExternalInput").ap()
    w_d = nc.dram_tensor("w", [O_SH, IN], F32, kind="ExternalInput").ap()
    la_d = nc.dram_tensor("la", [O_SH, IN], F32, kind="ExternalInput").ap()
    bias_d = nc.dram_tensor("bias", [O_SH], F32, kind="ExternalInput").ap()
    out_d = nc.dram_tensor("out", [B, O_SH], F32, kind="ExternalOutput").ap()

    with TileCtx(nc) as tc, ExitStack() as ctx:
        _kernel_body(ctx, tc, x_d, u_d, w_d, la_d, bias_d, out_d)

    nc.compile()
    _CACHE[key] = nc
    return nc


def TileCtx(nc):
    return tile.TileContext(nc)


def _bcast_row(ap_row):
    """[1, n] AP -> [128, n] AP with 0 partition stride."""
    return bass.AP(
        tensor=ap_row.tensor,
        offset=ap_row.offset,
        ap=[[0, 128], list(ap_row.ap[-1])],
    )


def _kernel_body(ctx, tc, x_d, u_d, w_d, la_d, bias_d, out_d):
    nc = tc.nc
    Ln = mybir.ActivationFunctionType.Ln
    Sig = mybir.ActivationFunctionType.Sigmoid
    op = mybir.AluOpType

    singles = ctx.enter_context(tc.tile_pool(name="singles", bufs=1))

    # --- constants: W, la as f16 [128, H, IN]; bias cols; x16 rows ---
    w16 = singles.tile([128, H, IN], F16)
    la16 = singles.tile([128, H, IN], F16)
    with tc.tile_pool(name="setup", bufs=1) as setup:
        w32 = setup.tile([128, H, IN], F32)
        nc.sync.dma_start(out=w32, in_=w_d.rearrange("(h p) i -> p h i", p=128))
        nc.vector.tensor_copy(w16, w32)
        la32 = setup.tile([128, H, IN], F32)
        nc.sync.dma_start(out=la32, in_=la_d.rearrange("(h p) i -> p h i", p=128))
        nc.vector.tensor_copy(la16, la32)

    x16_hbm = nc.dram_tensor("x16tmp", [B, IN], F16, kind="Internal").ap()
    with tc.tile_pool(name="setup2", bufs=1) as setup:
        x32 = setup.tile([B, IN], F32)
        nc.sync.dma_start(out=x32, in_=x_d)
        x16 = setup.tile([B, IN], F16)
        nc.vector.tensor_copy(x16, x32)
        nc.sync.dma_start(out=x16_hbm, in_=x16)

    bias_col = singles.tile([128, H], F32)
    nc.sync.dma_start(out=bias_col, in_=bias_d.rearrange("(h p) -> p h", p=128))

    # --- pools for the main loop ---
    upool = ctx.enter_context(tc.tile_pool(name="u", bufs=2))
    l1pool = ctx.enter_context(tc.tile_pool(name="l1", bufs=2))
    l2pool = ctx.enter_context(tc.tile_pool(name="l2", bufs=2))
    tpool = ctx.enter_context(tc.tile_pool(name="t", bufs=B_GROUP + 1))
    zpool = ctx.enter_context(tc.tile_pool(name="z", bufs=3))
    xbpool = ctx.enter_context(tc.tile_pool(name="xb", bufs=3))
    ppool = ctx.enter_context(tc.tile_pool(name="p", bufs=4))
    apool = ctx.enter_context(tc.tile_pool(name="acc", bufs=8))

    out_v = out_d.rearrange("b (h p) -> b p h", p=128)

    for g0 in range(0, B, B_GROUP):
        grp = range(g0, min(g0 + B_GROUP, B))
        t_tiles = {}
        # ---- phase 1: natural_log table set ----
        for b in grp:
            ut = upool.tile([128, H, IN], F32)
            nc.sync.dma_start(
                out=ut, in_=u_d[b].rearrange("(h p) i -> p h i", p=128)
            )
            l1 = l1pool.tile([128, H, IN], F16)
            nc.scalar.activation(l1, ut, Ln)                      # ln(u)
            l2 = l2pool.tile([128, H, IN], F16)
            nc.scalar.activation(l2, ut, Ln, bias=1.0, scale=-1.0)  # ln(1-u)
            nc.vector.tensor_sub(l1, l1, l2)                      # logit(u), in place
            t16 = tpool.tile([128, H, IN], F16)
            nc.gpsimd.tensor_add(t16, l1, la16)                   # + log_alpha
            t_tiles[b] = t16
        # ---- phase 2: sigmoid table set ----
        for b in grp:
            t16 = t_tiles[b]
            nc.scalar.activation(t16, t16, Sig, scale=1.5)        # s, in place
            z16 = zpool.tile([128, H, IN], F16)
            nc.vector.tensor_scalar(z16, t16, 1.2, -0.1, op.mult, op.add)
            nc.vector.tensor_scalar(z16, z16, 0.0, 1.0, op.max, op.min)
            xb = xbpool.tile([128, IN], F16)
            nc.sync.dma_start(out=xb, in_=_bcast_row(x16_hbm[b : b + 1, :]))
            for h in range(H):
                p16 = ppool.tile([128, IN], F16)
                nc.vector.tensor_mul(p16, z16[:, h, :], xb)
                acc = apool.tile([128, 1], F32)
                nc.vector.scalar_tensor_tensor(
                    out=p16,
                    in0=p16,
                    scalar=1.0,
                    in1=w16[:, h, :],
                    op0=op.bypass,
                    op1=op.mult,
                    accum_out=acc,
                )
                nc.vector.tensor_add(acc, acc, bias_col[:, h : h + 1])
                nc.sync.dma_start(out=out_v[b, :, h : h + 1], in_=acc)


def kernel(x, u, weight, log_alpha, bias):
    x = np.ascontiguousarray(x, dtype=np.float32)
    u = np.ascontiguousarray(u, dtype=np.float32)
    weight = np.ascontiguousarray(weight, dtype=np.float32)
    log_alpha = np.ascontiguousarray(log_alpha, dtype=np.float32)
    bias = np.ascontiguousarray(bias, dtype=np.float32)

    nc = _build_nc()

    in_maps = []
    for c in range(N_CORES):
        sl = slice(c * O_SH, (c + 1) * O_SH)
        in_maps.append(
            {
                "x": x,
                "u": np.ascontiguousarray(u[:, sl, :]),
                "w": np.ascontiguousarray(weight[sl]),
                "la": np.ascontiguousarray(log_alpha[sl]),
                "bias": np.ascontiguousarray(bias[sl]),
            }
        )

    import os

    trace = bool(int(os.environ.get("KERNEL_TRACE", "0")))
    res = run_bass_kernel_spmd(
        nc, in_maps, core_ids=list(range(N_CORES)), trace=trace
    )
    kernel._last = res

    out = np.empty((B, OUT), dtype=np.float32)
    for c in range(N_CORES):
        out[:, c * O_SH : (c + 1) * O_SH] = res.results[c]["out"]
    return out



# revision 4
# speedup vs baseline: 1.1874x; 1.1874x over previous
"""Trainium2 Bass kernel for L0-regularized linear forward (hard-concrete gate).

Computes out[b,o] = sum_i x[b,i] * W[o,i] * z[b,o,i] + bias[o]
  where s = sigmoid((log(u) - log1p(-u) + log_alpha) / (2/3))
        z = clip(s * 1.2 - 0.1, 0, 1)

Key algebra: with d = log_alpha - ln((1-u)/u), we have s = sigmoid(1.5*d),
and the [0,1] clip on z is EXACTLY a clamp of d to +-ln(11)/1.5 (since
1.2*sigmoid(ln 11) - 0.1 = 1). So
    z = 1.2 * sigmoid(1.5 * clamp(d, -L, L)) - 0.1,  L = ln(11)/1.5
and out[b,o] = 1.2 * sum_i x*W*sc - 0.1 * R[b,o] + bias[o], with
R = x @ W^T (computed once on the TensorEngine) and sc the clamped sigmoid.

Sharding: output-dim sharded, 2048/8 = 256 rows per core.

Per-core engine placement (the stream is 32 x [128part, 2, 2048] tiles):
  two routes per batch, mixed to balance ACT vs DVE busy time:
  route L (22 batches): ACT ln(u), ACT ln(1-u); POOL adds log_alpha;
                        DVE subtracts + clamps.
  route R (10 batches): DVE reciprocal 1/u (in place, fp32);
                        ACT ln(1/u - 1); DVE subtracts from log_alpha + clamps.
  both:                 ACT sigmoid(1.5*dc); DVE: *W (f16 2x), *x-broadcast
                        (f16 2x), then tensor_scalar(mult 1.2) with accum_out
                        (4x mode) for the i-reduction.
ACT table sets (natural_log vs sigmoid) are batched in groups of B_GROUP
batches to amortize the ~2.7us table switch.
"""

import sys
from contextlib import ExitStack

import numpy as np

if "/opt/trn_rl_repo" not in sys.path:
    sys.path.insert(0, "/opt/trn_rl_repo")

import concourse.bass as bass
import concourse.tile as tile
from concourse import bacc, mybir
from concourse.bass_utils import run_bass_kernel_spmd

F32 = mybir.dt.float32
F16 = mybir.dt.float16

B, OUT, IN = 32, 2048, 2048
N_CORES = 8
O_SH = OUT // N_CORES          # 256 output rows per core
H = O_SH // 128                # 2 partition-halves per core
B_GROUP = 8                    # batches per ACT-table-set phase
CLAMP_L = float(np.log(11.0) / 1.5)

# batches that take the DVE-reciprocal route (skips one ACT ln pass)
ROUTE_R = frozenset(b for b in range(0, 30, 3))  # 10 batches

_CACHE = {}


def _build_nc(trace=False):
    key = ("nc", trace)
    if key in _CACHE:
        return _CACHE[key]

    nc = bacc.Bacc(
        "TRN2",
        target_bir_lowering=False,
        debug=False,
        num_devices=N_CORES,
    )
    u_d = nc.dram_tensor("u", [B, O_SH, IN], F32, kind="ExternalInput").ap()
    la16_d = nc.dram_tensor("la16", [O_SH, IN], F16, kind="ExternalInput").ap()
    w16_d = nc.dram_tensor("w16", [O_SH, IN], F16, kind="ExternalInput").ap()
    x_d = nc.dram_tensor("x", [B, IN], F32, kind="ExternalInput").ap()
    wt_d = nc.dram_tensor("wt", [IN, O_SH], F32, kind="ExternalInput").ap()
    xt_d = nc.dram_tensor("xt", [IN, B], F32, kind="ExternalInput").ap()
    bias_d = nc.dram_tensor("bias", [O_SH], F32, kind="ExternalInput").ap()
    out_d = nc.dram_tensor("out", [B, O_SH], F32, kind="ExternalOutput").ap()

    with tile.TileContext(nc) as tc, ExitStack() as ctx:
        _kernel_body(ctx, tc, u_d, la16_d, w16_d, x_d, wt_d, xt_d, bias_d, out_d)

    nc.compile()
    _CACHE[key] = nc
    return nc


def _bcast_row(ap_row):
    """[1, n] AP -> [128, n] AP with 0 partition stride (DMA source only)."""
    return bass.AP(
        tensor=ap_row.tensor,
        offset=ap_row.offset,
        ap=[[0, 128], list(ap_row.ap[-1])],
    )


def _kernel_body(ctx, tc, u_d, la16_d, w16_d, x_d, wt_d, xt_d, bias_d, out_d):
    nc = tc.nc
    Ln = mybir.ActivationFunctionType.Ln
    Sig = mybir.ActivationFunctionType.Sigmoid
    op = mybir.AluOpType

    singles = ctx.enter_context(tc.tile_pool(name="singles", bufs=1))

    # --- constants ---
    w16sb = singles.tile([128, H, IN], F16)
    nc.sync.dma_start(out=w16sb, in_=w16_d.rearrange("(h p) i -> p h i", p=128))
    la16sb = singles.tile([128, H, IN], F16)
    nc.sync.dma_start(out=la16sb, in_=la16_d.rearrange("(h p) i -> p h i", p=128))
    bias_col = singles.tile([128, H], F32)
    nc.sync.dma_start(out=bias_col, in_=bias_d.rearrange("(h p) -> p h", p=128))
    negone = singles.tile([128, 1], F32)
    nc.vector.memset(negone, -1.0)
    C32 = singles.tile([128, H, B], F32)
    accv = singles.tile([128, H, B], F32)

    # x as f16 in HBM for per-batch partition-broadcast loads
    x16_hbm = nc.dram_tensor("x16tmp", [B, IN], F16, kind="Internal").ap()
    with tc.tile_pool(name="setup_x", bufs=1) as setup:
        x32 = setup.tile([B, IN], F32)
        nc.sync.dma_start(out=x32, in_=x_d)
        x16 = setup.tile([B, IN], F16)
        nc.vector.tensor_copy(x16, x32)
        nc.sync.dma_start(out=x16_hbm, in_=x16)

    # --- R = x @ W^T on the TensorEngine; C = bias - 0.1 R ---
    with tc.tile_pool(name="setup_r", bufs=1) as setup, tc.tile_pool(
        name="setup_psum", bufs=2, space="PSUM"
    ) as pp:
        wt32 = setup.tile([128, IN // 128, O_SH], F32)
        nc.sync.dma_start(out=wt32, in_=wt_d.rearrange("(ic p) o -> p ic o", p=128))
        xt32 = setup.tile([128, IN // 128, B], F32)
        nc.sync.dma_start(out=xt32, in_=xt_d.rearrange("(ic p) b -> p ic b", p=128))
        for oc in range(H):
            ps = pp.tile([128, B], F32)
            n_ic = IN // 128
            for ic in range(n_ic):
                nc.tensor.matmul(
                    ps,
                    wt32[:, ic, oc * 128 : (oc + 1) * 128],
                    xt32[:, ic, :],
                    start=(ic == 0),
                    stop=(ic == n_ic - 1),
                )
            # C[:, oc, :] = bias - 0.1 * R
            nc.vector.tensor_scalar(
                C32[:, oc, :], ps, -0.1, bias_col[:, oc : oc + 1], op.mult, op.add
            )

    # --- pools for the main loop ---
    upool = ctx.enter_context(tc.tile_pool(name="u", bufs=2))
    l1pool = ctx.enter_context(tc.tile_pool(name="l1", bufs=2))
    l2pool = ctx.enter_context(tc.tile_pool(name="l2", bufs=2))
    dcpool = ctx.enter_context(tc.tile_pool(name="dc", bufs=B_GROUP + 1))
    spool = ctx.enter_context(tc.tile_pool(name="s", bufs=2))
    xbpool = ctx.enter_context(tc.tile_pool(name="xb", bufs=2))
    ppool = ctx.enter_context(tc.tile_pool(name="p", bufs=3))

    for g0 in range(0, B, B_GROUP):
        grp = range(g0, min(g0 + B_GROUP, B))
        dc_tiles = {}
        # ---- phase A: natural_log table set ----
        for b in grp:
            ut = upool.tile([128, H, IN], F32)
            nc.sync.dma_start(
                out=ut, in_=u_d[b].rearrange("(h p) i -> p h i", p=128)
            )
            dc = dcpool.tile([128, H, IN], F16)
            if b in ROUTE_R:
                with nc.allow_low_precision(reason="recip feeds ln; f16 stream"):
                    nc.vector.reciprocal(ut, ut)                  # v = 1/u
                nc.scalar.activation(dc, ut, Ln, bias=negone)     # t = ln(v-1)
                nc.vector.tensor_tensor(
                    out=dc, in0=la16sb, in1=dc, op=op.subtract
                )                                                 # d = la - t
            else:
                l1 = l1pool.tile([128, H, IN], F16)
                nc.scalar.activation(l1, ut, Ln)                  # ln(u)
                l2 = l2pool.tile([128, H, IN], F16)
                nc.scalar.activation(l2, ut, Ln, bias=1.0, scale=-1.0)  # ln(1-u)
                nc.gpsimd.tensor_add(l1, l1, la16sb)              # g = la + ln(u)
                nc.vector.tensor_tensor(
                    out=dc, in0=l1, in1=l2, op=op.subtract
                )                                                 # d = g - ln(1-u)
            nc.vector.tensor_scalar(
                dc, dc, -CLAMP_L, CLAMP_L, op.max, op.min
            )                                                     # clamp
            dc_tiles[b] = dc
        # ---- phase B: sigmoid table set ----
        for b in grp:
            dc = dc_tiles[b]
            s16 = spool.tile([128, H, IN], F16)
            nc.scalar.activation(s16, dc, Sig, scale=1.5)         # sc
            nc.vector.tensor_tensor(out=s16, in0=s16, in1=w16sb, op=op.mult)
            xb = xbpool.tile([128, IN], F16)
            nc.sync.dma_start(out=xb, in_=_bcast_row(x16_hbm[b : b + 1, :]))
            for h in range(H):
                p16 = ppool.tile([128, IN], F16)
                nc.vector.tensor_tensor(
                    out=p16, in0=s16[:, h, :], in1=xb, op=op.mult
                )
                nc.vector.tensor_scalar(
                    p16, p16, 1.2, 0.0, op.mult, op.add,
                    accum_out=accv[:, h, b : b + 1],
                )

    # --- final combine + store ---
    outsb = singles.tile([128, H, B], F32)
    nc.vector.tensor_tensor(out=outsb, in0=accv, in1=C32, op=op.add)
    out_v = out_d.rearrange("b (h p) -> p h b", p=128)
    for h in range(H):
        nc.sync.dma_start(out=out_v[:, h, :], in_=outsb[:, h, :])


def kernel(x, u, weight, log_alpha, bias):
    x = np.ascontiguousarray(x, dtype=np.float32)
    u = np.ascontiguousarray(u, dtype=np.float32)
    weight = np.ascontiguousarray(weight, dtype=np.float32)
    log_alpha = np.ascontiguousarray(log_alpha, dtype=np.float32)
    bias = np.ascontiguousarray(bias, dtype=np.float32)

    nc = _build_nc()

    xt = np.ascontiguousarray(x.T)
    in_maps = []
    for c in range(N_CORES):
        sl = slice(c * O_SH, (c + 1) * O_SH)
        w_sl = weight[sl]
        in_maps.append(
            {
                "u": np.ascontiguousarray(u[:, sl, :]),
                "la16": np.ascontiguousarray(log_alpha[sl]).astype(np.float16),
                "w16": np.ascontiguousarray(w_sl).astype(np.float16),
                "x": x,
                "wt": np.ascontiguousarray(w_sl.T),
                "xt": xt,
                "bias": np.ascontiguousarray(bias[sl]),
            }
        )

    import os

    trace = bool(int(os.environ.get("KERNEL_TRACE", "0")))
    res = run_bass_kernel_spmd(
        nc, in_maps, core_ids=list(range(N_CORES)), trace=trace
    )
    kernel._last = res

    out = np.empty((B, OUT), dtype=np.float32)
    for c in range(N_CORES):
        out[:, c * O_SH : (c + 1) * O_SH] = res.results[c]["out"]
    return out
